# revision 11
# baseline (speedup 1.0000x reference)
"""Trainium2 Bass kernel for nn_MultiHeadAttention_61546881352366.

The reference module's observable output is NOT attention: the attention
result is dead code in the original torch module.  The output is

    out = fc0(concat_h(v @ Wv_h^T)) = (v @ Wcat^T) @ W0^T + b0

with Wcat = Wv.reshape(H*D, C); the two linear maps fuse into one:
out = v @ (W0 @ Wcat)^T + b0, a single [B*T, C] @ [C, C] matmul + bias.
k and q are unused.  Sharding: data-parallel over batch (B == 8 ==
n_cores); each core computes one [2048, 1024] @ [1024, 1024] product.

MIXED PRECISION: steady row tiles m4-m14 run contraction k-tiles 0-5 in
bf16 (78.6 TF/s) and k-tiles 6,7 as ONE fp8e4 DoubleRow pass (157 TF/s,
K=256/instr: 128 partitions x 2 packed sub-rows) -> PE time -4.8us and
~0.4MB less input DMA.  Predicted rel err 1.35e-2 vs the 2e-2 gate
(fp8 on all K would be 3.19e-2: fails).  e4m3's narrow normal range
cannot hold v (std 1) and Wc (std 0.03) at scales whose product is 1,
so the fp8 factors use optimal scales (v*16, w*512 -> product 8192) and
the bf16 factors are scaled by exact powers of two (v*64, w*128 ->
also 8192); everything accumulates at one consistent scale and drains
divide by 8192 in the same op as the bias add.

ISA constraint: a DoubleRow dst must START AT PARTITION 0 (verifier
check_dual_fp8_restriction), so the fp8 result for row-half h cannot
accumulate in place into rows 64-127 of the main PSUM bank.  Instead
each (h, j) half lands in a fresh [64, 512] psum tile; GPSIMD (idle
otherwise) computes s8 = ps8*PSC + bias into SBUF (no RMW on main
banks, so no PE<->gpsimd serialization), and the vector drain does
ob[half] = main[half]*PSC + s8 -- same total vector work as a plain
bias add.  PSUM pools are split 6 (main, 3 tiles in flight) + 2 (ps8
ping-pong); the fp8 halves are emitted MID-tile (h0, bf16 k0-2, h1,
bf16 k3-5) so every ps8 slot has a ~1.3us bf16 gap to be merged in
before reuse -> no PE stalls.

Schedule facts (from perfetto): the runtime preamble runs ~0-5.8us and
engine BODY work cannot start before ~6.9-7.2us; a dma_start DIRECT2D
costs ~0.65us of sequencer issue; queue data starts ~1.5us after the
first trigger; early DMA supply is ~350GB/s and descriptor-count bound
(128 descriptors per [128,X] tile regardless of X).  Schedule:

  - Warmup matmuls ramp the PE DVFS clock (PE runs at ~1.2GHz until
    ~3us of continuous activity; the real stream starts data-bound at
    ~10.4us); they read an UNINITIALIZED raw sbuf tensor (outside the
    tile pools) so they have zero deps and start at tensor body entry
    (~7.3us).
  - The first real matmul needs only w0a [128,512] + v0k0 [128,256]
    (192KB): these ride the SYNC hwdge queue, which strictly preempts
    the scalar queue on the shared DMA engines -- sync carries ONLY
    them (bulk on sync starves scalar's whole stream and queue-full
    backpressure stalls scalar's trigger issue, both measured).
  - Everything else rides scalar in exact consumption order.  Fill is
    k-outer over m0-2 (6 main banks), pure bf16 full-K; m3 and m15 are
    also pure bf16 (m15 keeps the simple drain tail).
  - Tail: m15 drains j0 early, then j1 in two 256-wide banks so only a
    [128,256] STT + 64KB DMA trails the final matmul; the two sliver
    DMAs ride the idle SYNC queue.
  - Output is written bf16 (host upconverts; halves output DMA).

NOTE: the core clock varies run to run (216 vs 259 ns/matmul states,
~+-20%); compare kernels via the modal TensorMatrix slice duration.
"""

import numpy as np

import concourse.bacc as bacc
import concourse.mybir as mybir
from concourse.tile import TileContext
from concourse.bass_utils import run_bass_kernel_spmd

B, T, C = 8, 2048, 1024
H, D = 16, 64
P = 128
KT = C // P       # 8 contraction k-tiles
KB = 6            # fp8 tiles: k 0..5 bf16, k 6,7 in one DoubleRow pass
MT = T // P       # 16 row tiles per core
MP = MT // 2      # 8 v pair strips
TV = 2 * P        # 256 tokens per v strip
NF = 512          # bf16 matmul moving free dim (= one PSUM bank of fp32)
NJ = C // NF      # 2 output column tiles
NQ = 256          # fp8 DoubleRow moving free dim (rhs free = 2*NQ = 512)
M8LO, M8HI = 4, 14  # row tiles m4..m14 use the fp8 path

_FP32 = mybir.dt.float32
_BF16 = mybir.dt.bfloat16
_FP8 = mybir.dt.float8e4
_DR = mybir.MatmulPerfMode.DoubleRow

SV8, SW8 = 16.0, 512.0          # fp8 factor scales
SVB, SWB = 64.0, 128.0          # bf16 factor scales (exact powers of 2)
PSC = 1.0 / (SV8 * SW8)         # psum scale (= 1/(SVB*SWB) too)

N_WARMUP = 7      # dummy matmuls bridging tensor-body start -> first data
G = 3             # fill-phase row tiles (k-outer, bounded by 6 main banks)


def _build():
    mul, add = mybir.AluOpType.mult, mybir.AluOpType.add
    nc = bacc.Bacc()
    w0a = nc.dram_tensor("w0a", [P, NF], _BF16, kind="ExternalInput")
    w0b = nc.dram_tensor("w0b", [P, NF], _BF16, kind="ExternalInput")
    wP = nc.dram_tensor("wP", [KT - 1, P, C], _BF16, kind="ExternalInput")
    w8 = nc.dram_tensor("w8", [P, 2, C], _FP8, kind="ExternalInput")
    v0k0 = nc.dram_tensor("v0k0", [P, TV], _BF16, kind="ExternalInput")
    v0k13 = nc.dram_tensor("v0k13", [P, 3 * TV], _BF16, kind="ExternalInput")
    v0k47 = nc.dram_tensor("v0k47", [P, 4 * TV], _BF16, kind="ExternalInput")
    v1 = nc.dram_tensor("v1", [P, KT * TV], _BF16, kind="ExternalInput")
    vQ = [
        nc.dram_tensor(f"vq{q}", [P, 2 * KB * TV], _BF16, kind="ExternalInput")
        for q in range(3)
    ]
    v8s = nc.dram_tensor("v8s", [P, 6 * 2 * TV], _FP8, kind="ExternalInput")
    v7k67 = nc.dram_tensor("v7k67", [P, 2 * TV], _BF16, kind="ExternalInput")
    bias = nc.dram_tensor("bias", [P, C], _BF16, kind="ExternalInput")
    out = nc.dram_tensor("out", [T, C], _BF16, kind="ExternalOutput")

    with TileContext(nc) as tc:
        with (
            tc.tile_pool(name="wpool", bufs=1) as wpool,
            tc.tile_pool(name="vpool", bufs=1) as vpool,
            tc.tile_pool(name="bpool", bufs=1) as bpool,
            tc.tile_pool(name="opool", bufs=6) as opool,
            tc.tile_pool(name="s8p", bufs=4) as s8p,
            tc.tile_pool(name="psm", bufs=6, space="PSUM") as psm,
            tc.tile_pool(name="ps8p", bufs=2, space="PSUM") as ps8p,
        ):
            # PE warmup: zero-dependency matmuls on an UNINITIALIZED raw
            # sbuf tensor; the product is garbage but ps_w is never read.
            scratch = nc.alloc_sbuf_tensor("warm_scratch", [P, NF], _BF16)
            ps_w = ps8p.tile([P, NF], _FP32, name="ps_w", tag="ps")
            for _ in range(N_WARMUP):
                nc.tensor.matmul(
                    ps_w, lhsT=scratch[:, :P], rhs=scratch[:, :],
                    start=True, stop=True,
                )

            w0a_sb = wpool.tile([P, NF], _BF16, name="w0a", tag="w0a")
            w0b_sb = wpool.tile([P, NF], _BF16, name="w0b", tag="w0b")
            v00_sb = vpool.tile([P, TV], _BF16, name="v00", tag="v00")
            v013_sb = vpool.tile([P, 3, TV], _BF16, name="v013", tag="v013")
            v047_sb = vpool.tile([P, 4, TV], _BF16, name="v047", tag="v047")
            v1_sb = vpool.tile([P, KT, TV], _BF16, name="v1", tag="v1")
            w8_sb = wpool.tile([P, 2, C], _FP8, name="w8", tag="w8")
            v8s_sb = vpool.tile([P, 6, 2, TV], _FP8, name="v8s", tag="v8s")
            v7_sb = vpool.tile([P, 2, TV], _BF16, name="v7k67", tag="v7k67")
            w_sb = [None] * KT

            def dma_w(k, eng):
                w_k = wpool.tile([P, C], _BF16, name=f"w_{k}", tag=f"w_{k}")
                eng.dma_start(out=w_k, in_=wP[k - 1])
                w_sb[k] = w_k

            # sync queue: ONLY the 192KB the first matmul needs
            nc.sync.dma_start(out=w0a_sb, in_=w0a[:, :])
            nc.sync.dma_start(out=v00_sb, in_=v0k0[:, :])
            # scalar queue, in exact consumption order
            nc.scalar.dma_start(out=w0b_sb, in_=w0b[:, :])
            nc.scalar.dma_start(out=v013_sb, in_=v0k13[:, :])
            dma_w(1, nc.scalar)
            dma_w(2, nc.scalar)
            nc.scalar.dma_start(out=v1_sb, in_=v1[:, :])
            nc.scalar.dma_start(out=v047_sb, in_=v0k47[:, :])
            dma_w(3, nc.scalar)
            dma_w(4, nc.scalar)
            dma_w(5, nc.scalar)
            dma_w(6, nc.scalar)
            dma_w(7, nc.scalar)
            b_sb = bpool.tile([P, C], _BF16, name="b_sb", tag="b_sb")
            nc.scalar.dma_start(out=b_sb, in_=bias[:, :])
            nc.scalar.dma_start(out=w8_sb, in_=w8[:, :])
            vq_sb = []
            for q in range(3):
                v_q = vpool.tile(
                    [P, 2, KB, TV], _BF16, name=f"vq_{q}", tag=f"vq_{q}"
                )
                nc.scalar.dma_start(out=v_q, in_=vQ[q][:, :])
                vq_sb.append(v_q)
                if q == 0:
                    nc.scalar.dma_start(out=v8s_sb, in_=v8s[:, :])
            nc.scalar.dma_start(out=v7_sb, in_=v7k67[:, :])

            def v_at(m, k):
                """bf16 lhsT slice [128(k-part), 128(m-rows)] for tile m."""
                mp, r = m // 2, m % 2
                sl = slice(r * P, (r + 1) * P)
                if mp == 0:
                    if k == 0:
                        return v00_sb[:, sl]
                    if k <= 3:
                        return v013_sb[:, k - 1, sl]
                    return v047_sb[:, k - 4, sl]
                if mp == 1:
                    return v1_sb[:, k, sl]
                if k >= KB:  # only m15 (strip 7) takes this path
                    return v7_sb[:, k - KB, sl]
                return vq_sb[(mp - 2) // 2][:, (mp - 2) % 2, k, sl]

            def v8_at(m, h):
                """fp8 lhsT [128, 2, 64] for tile m, row-half h."""
                mp, r = m // 2, m % 2
                sl = slice(r * P + h * 64, r * P + h * 64 + 64)
                return v8s_sb[:, mp - 2, :, sl]

            def w_at(k, j):
                if k == 0:
                    return (w0a_sb if j == 0 else w0b_sb)[:, :]
                return w_sb[k][:, j * NF : (j + 1) * NF]

            def mm(ps_mj, m, k, j, kl):
                nc.tensor.matmul(
                    ps_mj, lhsT=v_at(m, k), rhs=w_at(k, j),
                    start=(k == 0), stop=(k == kl),
                    skip_group_check=True,
                )

            def fp8_half(m, h, s8):
                """DoubleRow pass for k 768..1023, row-half h, both j.

                Each j lands in a fresh [64, NF] psum tile at partition 0
                (ISA: DoubleRow dst must start at partition 0); vector
                then computes s8[(h, j)] = ps8*PSC + bias into SBUF
                (gpsimd cannot access PSUM, per the BIR verifier)."""
                for j in range(NJ):
                    ps8 = ps8p.tile(
                        [64, NF], _FP32, name=f"ps8_{m}_{h}_{j}", tag="ps"
                    )
                    for q0 in range(0, NF, NQ):
                        cq = j * NF + q0
                        nc.tensor.matmul(
                            ps8[:, q0 : q0 + NQ],
                            lhsT=v8_at(m, h),
                            rhs=w8_sb[:, :, cq : cq + NQ],
                            start=True, stop=True,
                            perf_mode=_DR, skip_group_check=True,
                        )
                    st = s8p.tile(
                        [64, NF], _FP32, name=f"s8_{m}_{h}_{j}", tag="s8"
                    )
                    nc.vector.scalar_tensor_tensor(
                        st, ps8, PSC, b_sb[:64, j * NF : (j + 1) * NF],
                        op0=mul, op1=add,
                    )
                    s8[(h, j)] = st

            def drain_bf(m, ob, ps):
                """Drain a pure-bf16 tile: ob = ps*PSC + bias."""
                for j in range(NJ):
                    sl = slice(j * NF, (j + 1) * NF)
                    nc.vector.scalar_tensor_tensor(
                        ob[:, sl], ps[j], PSC, b_sb[:, sl], op0=mul, op1=add
                    )
                nc.scalar.dma_start(out=out[m * P : (m + 1) * P, :], in_=ob)

            def drain_fp8(m, ob, ps, s8):
                """Drain an fp8 tile: ob[half] = main[half]*PSC + s8."""
                for j in range(NJ):
                    sl = slice(j * NF, (j + 1) * NF)
                    for h in range(2):
                        hs = slice(h * 64, (h + 1) * 64)
                        nc.vector.scalar_tensor_tensor(
                            ob[hs, sl], ps[j][hs, :], PSC, s8[(h, j)],
                            op0=mul, op1=add,
                        )
                nc.scalar.dma_start(out=out[m * P : (m + 1) * P, :], in_=ob)

            # Fill phase (m0-2): k-outer full-K bf16, ordered to match
            # DMA arrival.
            psg = {
                (m, j): psm.tile([P, NF], _FP32, name=f"ps_{m}_{j}", tag="ps")
                for m in range(G)
                for j in range(NJ)
            }
            obg = {
                m: opool.tile([P, C], _BF16, name=f"ob_{m}", tag="ob")
                for m in range(G)
            }

            def fill(ms, ks, js=range(NJ)):
                for k in ks:
                    for m in ms:
                        for j in js:
                            mm(psg[m, j], m, k, j, KT - 1)
                        if k == KT - 1:
                            drain_bf(m, obg[m], [psg[m, j] for j in range(NJ)])

            fill((0, 1), (0,), js=(0,))   # needs w0a + v0k0 only
            fill((0, 1), (0,), js=(1,))   # + w0b
            fill((0, 1), (1,))            # + v0k13, w1
            fill((0, 1), (2,))            # + w2
            fill((2,), (0, 1, 2))         # + v1
            fill((0, 1, 2), range(3, KT))  # + v0k47, w3..w7

            # m3: pure bf16, m-major (v1 strip covers full K).
            m = G
            ob3 = opool.tile([P, C], _BF16, name="ob_3", tag="ob")
            ps3 = [
                psm.tile([P, NF], _FP32, name=f"ps_3_{j}", tag="ps")
                for j in range(NJ)
            ]
            for k in range(KT):
                for j in range(NJ):
                    mm(ps3[j], m, k, j, KT - 1)
            drain_bf(m, ob3, ps3)

            # Steady phase (m4-14): fp8 k67 + bf16 k0-5, fp8 halves
            # emitted mid-tile so ps8 slots merge during bf16 gaps.
            for m in range(M8LO, M8HI + 1):
                ob = opool.tile([P, C], _BF16, name=f"ob_{m}", tag="ob")
                ps = [
                    psm.tile([P, NF], _FP32, name=f"ps_{m}_{j}", tag="ps")
                    for j in range(NJ)
                ]
                s8 = {}
                fp8_half(m, 0, s8)
                for k in range(3):
                    for j in range(NJ):
                        mm(ps[j], m, k, j, KB - 1)
                fp8_half(m, 1, s8)
                for k in range(3, KB):
                    for j in range(NJ):
                        mm(ps[j], m, k, j, KB - 1)
                drain_fp8(m, ob, ps, s8)

            # Last m-tile (m15): pure bf16 full-K; j0 drains early, j1
            # in two 256-wide banks so only a [128,256] STT + 64KB DMA
            # trails the final matmul; sliver DMAs ride the SYNC queue.
            m = MT - 1
            ob = opool.tile([P, C], _BF16, name=f"ob_{m}", tag="ob")
            ps_j = psm.tile([P, NF], _FP32, name=f"ps_{m}_0", tag="ps")
            for k in range(KT):
                mm(ps_j, m, k, 0, KT - 1)
            sl = slice(0, NF)
            nc.vector.scalar_tensor_tensor(
                ob[:, sl], ps_j, PSC, b_sb[:, sl], op0=mul, op1=add
            )
            nc.scalar.dma_start(out=out[m * P : (m + 1) * P, sl], in_=ob[:, sl])
            for hh in range(2):
                ps_h = psm.tile([P, NQ], _FP32, name=f"ps_{m}_1{hh}", tag="ps")
                c0 = NF + hh * NQ
                sl = slice(c0, c0 + NQ)
                for k in range(KT):
                    nc.tensor.matmul(
                        ps_h,
                        lhsT=v_at(m, k),
                        rhs=w0b_sb[:, hh * NQ : (hh + 1) * NQ]
                        if k == 0
                        else w_sb[k][:, sl],
                        start=(k == 0), stop=(k == KT - 1),
                        skip_group_check=True,
                    )
                nc.vector.scalar_tensor_tensor(
                    ob[:, sl], ps_h, PSC, b_sb[:, sl], op0=mul, op1=add
                )
                nc.sync.dma_start(
                    out=out[m * P : (m + 1) * P, sl], in_=ob[:, sl]
                )
    nc.compile()
    return nc


_nc_cache = None


def _get_nc():
    global _nc_cache
    if _nc_cache is None:
        _nc_cache = _build()
    return _nc_cache


def prepare_inputs(inputs):
    """Host-side prep shared by kernel() and the timing harness."""
    import ml_dtypes

    v = np.ascontiguousarray(np.asarray(inputs["v"], dtype=np.float32))
    Wv = np.asarray(inputs["Wv"], dtype=np.float32)
    W0 = np.asarray(inputs["W0"], dtype=np.float32)
    b0 = np.asarray(inputs["b0"], dtype=np.float32)

    # Fuse the two linear layers on the host: Wc = W0 @ Wcat, [C_out, C_in]
    Wc = W0 @ Wv.reshape(H * D, C)
    CB = KB * P  # 768: first contraction column of the fp8 k-pair

    # --- bf16 factors, scaled by exact powers of two ---
    # wP[k, p, j] = SWB * Wc[j, k*128+p], all 8 k
    wPa = np.ascontiguousarray(
        (Wc.T * SWB).reshape(KT, P, C).astype(ml_dtypes.bfloat16)
    )
    w0a = np.ascontiguousarray(wPa[0][:, :NF])
    w0b = np.ascontiguousarray(wPa[0][:, NF:])
    wP_rest = np.ascontiguousarray(wPa[1:])
    # vP[b, mp, p, k, tt] = SVB * v[b, mp*256+tt, k*128+p]
    vb = (v * SVB).astype(ml_dtypes.bfloat16)
    vP = vb.reshape(B, MP, TV, KT, P).transpose(0, 1, 4, 3, 2)
    # strip 0 (m0/m1): split k0 / k1-3 / k4-7; strip 1 (m2/m3): full K
    v0 = vP[:, 0].reshape(B, P, KT * TV)
    v0k0 = np.ascontiguousarray(v0[:, :, :TV])
    v0k13 = np.ascontiguousarray(v0[:, :, TV : 4 * TV])
    v0k47 = np.ascontiguousarray(v0[:, :, 4 * TV :])
    v1 = np.ascontiguousarray(vP[:, 1].reshape(B, P, KT * TV))
    # strips 2-7 (m4-m15): k0-5 bf16 only, packed as three 2-strip chunks
    vq = [
        np.ascontiguousarray(
            vP[:, 2 + 2 * q : 4 + 2 * q, :, :KB].transpose(0, 2, 1, 3, 4)
            .reshape(B, P, 2 * KB * TV)
        )
        for q in range(3)
    ]
    # strip 7 k6,7 in bf16 for the pure-bf16 last tile (m15)
    v7k67 = np.ascontiguousarray(
        vP[:, 7, :, KB:].reshape(B, P, 2 * TV)
    )

    # --- fp8 factors (k-tiles 6,7 as one DoubleRow pair), m4-m14 ---
    # w8[p, i, j] = e4m3(SW8 * Wc[j, 768 + p + 128*i])
    w8 = np.ascontiguousarray(
        (Wc.T[CB:] * SW8).reshape(2, P, C).transpose(1, 0, 2)
        .astype(ml_dtypes.float8_e4m3)
    )
    # v8[b, s, p, i, t] = e4m3(SV8 * v[b, s*256+t, 768 + p + 128*i])
    v8 = (
        (v[..., CB:] * SV8)
        .reshape(B, MP, TV, 2, P)
        .transpose(0, 1, 4, 3, 2)
        .astype(ml_dtypes.float8_e4m3)
    )
    v8s = np.ascontiguousarray(
        v8[:, 2:].transpose(0, 2, 1, 3, 4).reshape(B, P, 6 * 2 * TV)
    )

    bias = np.ascontiguousarray(
        np.broadcast_to(b0[None, :], (P, C)).astype(ml_dtypes.bfloat16)
    )
    return [
        {
            "w0a": w0a,
            "w0b": w0b,
            "wP": wP_rest,
            "w8": w8,
            "v0k0": v0k0[i],
            "v0k13": v0k13[i],
            "v0k47": v0k47[i],
            "v1": v1[i],
            "vq0": vq[0][i],
            "vq1": vq[1][i],
            "vq2": vq[2][i],
            "v8s": v8s[i],
            "v7k67": v7k67[i],
            "bias": bias,
        }
        for i in range(B)
    ]


def kernel(**inputs):
    in_maps = prepare_inputs(inputs)
    nc = _get_nc()
    res = run_bass_kernel_spmd(nc, in_maps, core_ids=list(range(B)))
    return np.stack(
        [res.results[i]["out"].astype(np.float32) for i in range(B)], axis=0
    )


# revision 12
# speedup vs baseline: 1.2691x; 1.2691x over previous
"""Trainium2 Bass kernel for nn_MultiHeadAttention_61546881352366.

The reference module's observable output is NOT attention: the attention
result is dead code in the original torch module.  The output is

    out = fc0(concat_h(v @ Wv_h^T)) = (v @ Wcat^T) @ W0^T + b0

with Wcat = Wv.reshape(H*D, C); the two linear maps fuse into one:
out = v @ (W0 @ Wcat)^T + b0, a single [B*T, C] @ [C, C] matmul + bias.
k and q are unused.  Sharding: data-parallel over batch (B == 8 ==
n_cores); each core computes one [2048, 1024] @ [1024, 1024] bf16
product (fp32 PSUM accumulate; rel err ~2.9e-3 vs the 2e-2 gate).

bf16 is the right precision: fp8 e4m3 on all K fails the gate (3.2e-2
measured), and partial fp8 via DoubleRow was measured to give ZERO PE
win on this hardware -- the ISA forces the DoubleRow dst to partition 0
(M<=64), [64,256] fp8 matmuls run at ~109ns (1 cycle/row, not the cost
model's 0.5), so 2x K per pass x 0.5x M = 1x throughput, while the
[64,*] DVE merge ops cost full-width time (idle lanes) and made the
vector engine the bottleneck (93.9us total).

Timeline facts (from perfetto): the runtime preamble runs ~0-5.8us and
engine BODY work cannot start before ~6.9-7.2us; a dma_start DIRECT2D
costs ~0.65us of sequencer issue; queue data starts ~1.5us after the
first trigger; early DMA supply is ~350GB/s and descriptor-count bound
(128 descriptors per [128,X] tile regardless of X); supply caps how
early the matmul stream can run, so the schedule targets a gapless PE
stream from the earliest supply-feasible start (~10.5us):

  - Warmup matmuls ramp the PE DVFS clock (PE runs at ~0.65-1.2GHz
    until ~3us of continuous activity and 216ns/matmul after); they
    read an UNINITIALIZED raw sbuf tensor (outside the tile pools) so
    they have zero deps and start at tensor body entry (~7.3us), and
    N_WARMUP=7 bridges exactly to first-data (~10.4us) -- a gap here
    resets the ramp and cost v3 ~2.3us of mid-pstate real matmuls.
  - The first real matmul needs only w0a [128,512] + v0k0 [128,256]
    (192KB): these ride the SYNC hwdge queue, which strictly preempts
    the scalar queue on the shared DMA engines -- sync carries ONLY
    them (bulk on sync starves scalar's whole stream and queue-full
    backpressure stalls scalar's trigger issue, both measured).
  - Everything else rides scalar in exact consumption order; fill is
    k-outer over m0-3 (8 PSUM banks), ordered to match arrival:
    (m01 k0 j0), (m01 k0 j1), (m01 k1), (m01 k2), (m23 k0-2),
    (m0-3 k3..k7).
  - Factors are scaled by exact powers of two (v*64, w*128; lossless
    in bf16) and the drain applies (psum/8192 + bias) with a single
    vector scalar_tensor_tensor op -- same cost as a plain bias add.
    (Kept from the fp8 experiments; harmless, and lets a future mixed
    path accumulate consistently.)
  - Tail: m15 drains j0 early, then j1 in two 256-wide banks so only a
    [128,256] STT + 64KB DMA trails the final matmul; the two sliver
    DMAs ride the idle SYNC queue so they don't wait behind the
    m14/m15j0 drains on the scalar queue.
  - Output is written bf16 (host upconverts; halves output DMA).

NOTE: the core clock varies run to run (216 vs 259 ns/matmul states,
~+-20%); compare kernels via the modal TensorMatrix slice duration.
"""

import numpy as np

import concourse.bacc as bacc
import concourse.mybir as mybir
from concourse.tile import TileContext
from concourse.bass_utils import run_bass_kernel_spmd

B, T, C = 8, 2048, 1024
H, D = 16, 64
P = 128
KT = C // P       # 8 contraction k-tiles
MT = T // P       # 16 row tiles per core
MP = MT // 2      # 8 v pair strips
TV = 2 * P        # 256 tokens per v strip
NF = 512          # matmul moving free dim (= one PSUM bank of fp32)
NJ = C // NF      # 2 output column tiles

_FP32 = mybir.dt.float32
_BF16 = mybir.dt.bfloat16

SVB, SWB = 64.0, 128.0          # bf16 factor scales (exact powers of 2)
PSC = 1.0 / (SVB * SWB)         # psum scale

N_WARMUP = 7      # dummy matmuls bridging tensor-body start -> first data
G = 4             # fill-phase row tiles (k-outer, bounded by 8 PSUM banks)


def _build():
    mul, add = mybir.AluOpType.mult, mybir.AluOpType.add
    nc = bacc.Bacc()
    w0a = nc.dram_tensor("w0a", [P, NF], _BF16, kind="ExternalInput")
    w0b = nc.dram_tensor("w0b", [P, NF], _BF16, kind="ExternalInput")
    wP = nc.dram_tensor("wP", [KT - 1, P, C], _BF16, kind="ExternalInput")
    v0k0 = nc.dram_tensor("v0k0", [P, TV], _BF16, kind="ExternalInput")
    v0k13 = nc.dram_tensor("v0k13", [P, 3 * TV], _BF16, kind="ExternalInput")
    v0k47 = nc.dram_tensor("v0k47", [P, 4 * TV], _BF16, kind="ExternalInput")
    v1 = nc.dram_tensor("v1", [P, KT * TV], _BF16, kind="ExternalInput")
    vQ = [
        nc.dram_tensor(f"vq{q}", [P, 2 * KT * TV], _BF16, kind="ExternalInput")
        for q in range(3)
    ]
    bias = nc.dram_tensor("bias", [P, C], _BF16, kind="ExternalInput")
    out = nc.dram_tensor("out", [T, C], _BF16, kind="ExternalOutput")

    with TileContext(nc) as tc:
        with (
            tc.tile_pool(name="wpool", bufs=1) as wpool,
            tc.tile_pool(name="vpool", bufs=1) as vpool,
            tc.tile_pool(name="bpool", bufs=1) as bpool,
            tc.tile_pool(name="opool", bufs=6) as opool,
            tc.tile_pool(name="pspool", bufs=8, space="PSUM") as pspool,
        ):
            # PE warmup: zero-dependency matmuls on an UNINITIALIZED raw
            # sbuf tensor; the product is garbage but ps_w is never read.
            scratch = nc.alloc_sbuf_tensor("warm_scratch", [P, NF], _BF16)
            ps_w = pspool.tile([P, NF], _FP32, name="ps_w", tag="ps")
            for _ in range(N_WARMUP):
                nc.tensor.matmul(
                    ps_w, lhsT=scratch[:, :P], rhs=scratch[:, :],
                    start=True, stop=True,
                )

            w0a_sb = wpool.tile([P, NF], _BF16, name="w0a", tag="w0a")
            w0b_sb = wpool.tile([P, NF], _BF16, name="w0b", tag="w0b")
            v00_sb = vpool.tile([P, TV], _BF16, name="v00", tag="v00")
            v013_sb = vpool.tile([P, 3, TV], _BF16, name="v013", tag="v013")
            v047_sb = vpool.tile([P, 4, TV], _BF16, name="v047", tag="v047")
            v1_sb = vpool.tile([P, KT, TV], _BF16, name="v1", tag="v1")
            w_sb = [None] * KT

            def dma_w(k, eng):
                w_k = wpool.tile([P, C], _BF16, name=f"w_{k}", tag=f"w_{k}")
                eng.dma_start(out=w_k, in_=wP[k - 1])
                w_sb[k] = w_k

            # sync queue: ONLY the 192KB the first matmul needs
            nc.sync.dma_start(out=w0a_sb, in_=w0a[:, :])
            nc.sync.dma_start(out=v00_sb, in_=v0k0[:, :])
            # scalar queue, in exact consumption order
            nc.scalar.dma_start(out=w0b_sb, in_=w0b[:, :])
            nc.scalar.dma_start(out=v013_sb, in_=v0k13[:, :])
            dma_w(1, nc.scalar)
            dma_w(2, nc.scalar)
            nc.scalar.dma_start(out=v1_sb, in_=v1[:, :])
            nc.scalar.dma_start(out=v047_sb, in_=v0k47[:, :])
            dma_w(3, nc.scalar)
            dma_w(4, nc.scalar)
            dma_w(5, nc.scalar)
            dma_w(6, nc.scalar)
            dma_w(7, nc.scalar)
            b_sb = bpool.tile([P, C], _BF16, name="b_sb", tag="b_sb")
            nc.scalar.dma_start(out=b_sb, in_=bias[:, :])
            vq_sb = []
            for q in range(3):
                v_q = vpool.tile(
                    [P, 2, KT, TV], _BF16, name=f"vq_{q}", tag=f"vq_{q}"
                )
                nc.scalar.dma_start(out=v_q, in_=vQ[q][:, :])
                vq_sb.append(v_q)

            def v_at(m, k):
                """lhsT slice [128(k-part), 128(m-rows)] for row tile m."""
                mp, r = m // 2, m % 2
                sl = slice(r * P, (r + 1) * P)
                if mp == 0:
                    if k == 0:
                        return v00_sb[:, sl]
                    if k <= 3:
                        return v013_sb[:, k - 1, sl]
                    return v047_sb[:, k - 4, sl]
                if mp == 1:
                    return v1_sb[:, k, sl]
                return vq_sb[(mp - 2) // 2][:, (mp - 2) % 2, k, sl]

            def w_at(k, j):
                if k == 0:
                    return (w0a_sb if j == 0 else w0b_sb)[:, :]
                return w_sb[k][:, j * NF : (j + 1) * NF]

            def mm(ps_mj, m, k, j):
                nc.tensor.matmul(
                    ps_mj, lhsT=v_at(m, k), rhs=w_at(k, j),
                    start=(k == 0), stop=(k == KT - 1),
                )

            def drain(m, ob, ps):
                for j in range(NJ):
                    sl = slice(j * NF, (j + 1) * NF)
                    nc.vector.scalar_tensor_tensor(
                        ob[:, sl], ps[j], PSC, b_sb[:, sl], op0=mul, op1=add
                    )
                nc.scalar.dma_start(out=out[m * P : (m + 1) * P, :], in_=ob)

            # Fill phase (m0-3): k-outer, ordered to match DMA arrival.
            psg = {
                (m, j): pspool.tile([P, NF], _FP32, name=f"ps_{m}_{j}", tag="ps")
                for m in range(G)
                for j in range(NJ)
            }
            obg = {
                m: opool.tile([P, C], _BF16, name=f"ob_{m}", tag="ob")
                for m in range(G)
            }

            def fill(ms, ks, js=range(NJ)):
                for k in ks:
                    for m in ms:
                        for j in js:
                            mm(psg[m, j], m, k, j)
                        if k == KT - 1:
                            drain(m, obg[m], [psg[m, j] for j in range(NJ)])

            fill((0, 1), (0,), js=(0,))   # needs w0a + v0k0 only
            fill((0, 1), (0,), js=(1,))   # + w0b
            fill((0, 1), (1,))            # + v0k13, w1
            fill((0, 1), (2,))            # + w2
            fill((2, 3), (0, 1, 2))       # + v1
            fill((0, 1, 2, 3), range(3, KT))  # + v0k47, w3..w7

            # Steady phase (m4-14): m-major, copies pace with compute.
            for m in range(G, MT - 1):
                ob = opool.tile([P, C], _BF16, name=f"ob_{m}", tag="ob")
                ps = [
                    pspool.tile([P, NF], _FP32, name=f"ps_{m}_{j}", tag="ps")
                    for j in range(NJ)
                ]
                for k in range(KT):
                    for j in range(NJ):
                        mm(ps[j], m, k, j)
                drain(m, ob, ps)

            # Last m-tile: j0 drains early; j1 in two 256-wide banks so
            # only a [128,256] STT + 64KB DMA trails the final matmul;
            # the sliver DMAs ride the idle SYNC queue.
            m = MT - 1
            ob = opool.tile([P, C], _BF16, name=f"ob_{m}", tag="ob")
            ps_j = pspool.tile([P, NF], _FP32, name=f"ps_{m}_0", tag="ps")
            for k in range(KT):
                mm(ps_j, m, k, 0)
            sl = slice(0, NF)
            nc.vector.scalar_tensor_tensor(
                ob[:, sl], ps_j, PSC, b_sb[:, sl], op0=mul, op1=add
            )
            nc.scalar.dma_start(out=out[m * P : (m + 1) * P, sl], in_=ob[:, sl])
            half = NF // 2
            for hh in range(2):
                ps_h = pspool.tile([P, half], _FP32, name=f"ps_{m}_1{hh}", tag="ps")
                c0 = NF + hh * half
                sl = slice(c0, c0 + half)
                for k in range(KT):
                    nc.tensor.matmul(
                        ps_h,
                        lhsT=v_at(m, k),
                        rhs=w0b_sb[:, hh * half : (hh + 1) * half]
                        if k == 0
                        else w_sb[k][:, sl],
                        start=(k == 0), stop=(k == KT - 1),
                    )
                nc.vector.scalar_tensor_tensor(
                    ob[:, sl], ps_h, PSC, b_sb[:, sl], op0=mul, op1=add
                )
                nc.sync.dma_start(
                    out=out[m * P : (m + 1) * P, sl], in_=ob[:, sl]
                )
    nc.compile()
    return nc


_nc_cache = None


def _get_nc():
    global _nc_cache
    if _nc_cache is None:
        _nc_cache = _build()
    return _nc_cache


def prepare_inputs(inputs):
    """Host-side prep shared by kernel() and the timing harness."""
    import ml_dtypes

    v = np.ascontiguousarray(np.asarray(inputs["v"], dtype=np.float32))
    Wv = np.asarray(inputs["Wv"], dtype=np.float32)
    W0 = np.asarray(inputs["W0"], dtype=np.float32)
    b0 = np.asarray(inputs["b0"], dtype=np.float32)

    # Fuse the two linear layers on the host: Wc = W0 @ Wcat, [C_out, C_in]
    Wc = W0 @ Wv.reshape(H * D, C)
    # wP[k, p, j] = SWB * Wc[j, k*128+p]
    wPa = np.ascontiguousarray(
        (Wc.T * SWB).reshape(KT, P, C).astype(ml_dtypes.bfloat16)
    )
    w0a = np.ascontiguousarray(wPa[0][:, :NF])
    w0b = np.ascontiguousarray(wPa[0][:, NF:])
    wP_rest = np.ascontiguousarray(wPa[1:])
    # vP[b, mp, p, k*256+tt] = SVB * v[b, mp*256+tt, k*128+p]
    vb = (v * SVB).astype(ml_dtypes.bfloat16)
    vP = vb.reshape(B, MP, TV, KT, P).transpose(0, 1, 4, 3, 2).reshape(
        B, MP, P, KT * TV
    )
    v0k0 = np.ascontiguousarray(vP[:, 0, :, :TV])
    v0k13 = np.ascontiguousarray(vP[:, 0, :, TV : 4 * TV])
    v0k47 = np.ascontiguousarray(vP[:, 0, :, 4 * TV :])
    v1 = np.ascontiguousarray(vP[:, 1])
    vq = [
        np.ascontiguousarray(
            vP[:, 2 + 2 * q : 4 + 2 * q].transpose(0, 2, 1, 3).reshape(
                B, P, 2 * KT * TV
            )
        )
        for q in range(3)
    ]
    bias = np.ascontiguousarray(
        np.broadcast_to(b0[None, :], (P, C)).astype(ml_dtypes.bfloat16)
    )
    return [
        {
            "w0a": w0a,
            "w0b": w0b,
            "wP": wP_rest,
            "v0k0": v0k0[i],
            "v0k13": v0k13[i],
            "v0k47": v0k47[i],
            "v1": v1[i],
            "vq0": vq[0][i],
            "vq1": vq[1][i],
            "vq2": vq[2][i],
            "bias": bias,
        }
        for i in range(B)
    ]


def kernel(**inputs):
    in_maps = prepare_inputs(inputs)
    nc = _get_nc()
    res = run_bass_kernel_spmd(nc, in_maps, core_ids=list(range(B)))
    return np.stack(
        [res.results[i]["out"].astype(np.float32) for i in range(B)], axis=0
    )
